# revision 1
# baseline (speedup 1.0000x reference)
"""GAT-pooling segment-softmax kernel for 8 Trainium2 NeuronCores.

Math (matches the reference):
    feats  = relu(x @ W1.T + b1)          [E, H]
    scores = feats @ w2 + b2              [E]
    w      = segment_softmax(scores)      (per segment of sorted batch_indices)
    out[s] = sum_{i in seg s} w_i * x_i   [S, H]

Scores are O(1) here, so exp() without the per-segment max subtraction is
numerically safe; softmax normalization happens on the host from per-segment
partial sums: the device returns per-(core, supermacro, column) weighted sums
(numer) and the raw per-edge exp weights (e), the host reduces them.

Device layout (per core, SPMD — one program, per-core data):
  * edges are padded per-core to G supermacros of 4096 edges
    (32 subchunks x 128 partitions; edge = g*4096 + p*32 + q so each
    partition reads 8KB contiguous from DRAM)
  * per subchunk q: PE transposes xb -> xbT (psum), DVE copies to sbuf,
    PE feats = W1t.T @ xbT (bf16, psum f32), relu+b1 -> fr (ACT or DVE),
    PE scores col = fr_sub.T @ w2  ([128t, 1] psum)
  * per supermacro: ACT exp -> e (bf16, also DMA'd to host), DVE builds
    A[:, q, c] = mask_c * e, PE accumulates numer[c, :] += A[:, q, c].T @ xb_q
    into one [2, H] psum tile over all 32 subchunks.
  * host folds partials into [S, H]: numer from the device, denom = sum of
    masked e (bitwise-identical weights to the device's A since masks are 0/1).

A supermacro (4096 edges) can span at most 2 distinct segments whenever every
segment has >4096 edges (true for the target distribution); the host verifies
this and falls back to a pure-numpy path otherwise.
"""

import sys

sys.path.insert(0, "/opt/trn_rl_repo")

import ml_dtypes
import numpy as np

import concourse.bass as bass
import concourse.mybir as mybir
import concourse.tile as tile
from concourse.bass_utils import run_bass_kernel_spmd

NCORES = 8
H = 128
SUB = 128  # edges per subchunk (matmul contraction = partition dim)
QPG = 32  # subchunks per supermacro
SUPER = SUB * QPG  # 4096
MPG = 8  # macros (4 subchunks) per supermacro
NSEG = 256
SEGC = 2  # segment columns per supermacro (max distinct segments)
RELU_DVE_MOD = 1000000  # every Nth macro computes relu on DVE instead of ACT
MACRO_W = 2  # macros fused per transpose/feats/relu block (1 or 2)
NUMER_FLIP = True  # True: x as weights, A streams (2 cyc); False: A as weights
BUFS = {"xb": 4, "mk": 3, "xbt_ps": 2, "xbt_sb": 4, "feat_ps": 2, "fr": 4,
        "sc_ps": 1, "e": 3, "amat": 3, "nm_ps": 1}

BF16 = mybir.dt.bfloat16
F32 = mybir.dt.float32
AF = mybir.ActivationFunctionType
ALU = mybir.AluOpType


def _split_multi_waits(nc):
    """The walrus build in this container encodes at most one sync-wait per
    instruction; Tile emits several.  Spill extras onto standalone
    EventSemaphore instructions just before the gated instruction (same
    engine, so semantics are identical)."""
    for f in nc.m.functions:
        for b in f.blocks:
            insts = list(b.instructions)
            out = []
            changed = False
            for ins in insts:
                si = ins.sync_info
                waits = list(si.on_wait) if si else []
                if len(waits) > 1:
                    for k, w in enumerate(waits[1:]):
                        out.append(
                            mybir.InstEventSemaphore(
                                name=f"{ins.name}-wsplit{k}",
                                engine=ins.engine,
                                ins=[],
                                outs=[],
                                sync_info=mybir.SyncInfo(on_wait=[w], on_update=[]),
                            )
                        )
                    si.on_wait = waits[:1]
                    ins.sync_info = si
                    changed = True
                out.append(ins)
            if changed:
                b.instructions = out


def _build(G, b2_val, reps=1):
    """Build the single-core Bass program (shared verbatim by all 8 cores).

    reps>1 wraps the body in an on-device For_i loop re-running the whole
    kernel (same data) — used only for wall-clock benchmarking."""
    nc = bass.Bass()

    xb_d = nc.declare_dram_parameter("xb", [G, 128, QPG * H], BF16, isOutput=False)
    mk_d = nc.declare_dram_parameter("mk", [G, 128, QPG, SEGC], BF16, isOutput=False)
    w1t_d = nc.declare_dram_parameter("w1t", [H, H], BF16, isOutput=False)
    w2_d = nc.declare_dram_parameter("w2c", [H, 1], BF16, isOutput=False)
    b1_d = nc.declare_dram_parameter("b1c", [H, 1], F32, isOutput=False)
    id_d = nc.declare_dram_parameter("ident", [128, 128], BF16, isOutput=False)
    out_d = nc.declare_dram_parameter("partials", [128, G * SEGC], F32, isOutput=True)
    e_d = nc.declare_dram_parameter("eout", [G, 128, QPG], BF16, isOutput=True)

    with tile.TileContext(nc) as tc:
        with (
            tc.tile_pool(name="consts", bufs=1) as cpool,
            tc.tile_pool(name="xb", bufs=BUFS["xb"]) as xpool,
            tc.tile_pool(name="mk", bufs=BUFS["mk"]) as mpool,
            tc.tile_pool(name="xbt_ps", bufs=BUFS["xbt_ps"], space="PSUM") as tp_ps,
            tc.tile_pool(name="xbt_sb", bufs=BUFS["xbt_sb"]) as tp_sb,
            tc.tile_pool(name="feat_ps", bufs=BUFS["feat_ps"], space="PSUM") as fp_ps,
            tc.tile_pool(name="fr", bufs=BUFS["fr"]) as fr_pool,
            tc.tile_pool(name="sc_ps", bufs=BUFS["sc_ps"], space="PSUM") as sc_ps,
            tc.tile_pool(name="e", bufs=BUFS["e"]) as e_pool,
            tc.tile_pool(name="amat", bufs=BUFS["amat"]) as a_pool,
            tc.tile_pool(name="nm_ps", bufs=BUFS["nm_ps"], space="PSUM") as nm_ps,
            tc.tile_pool(name="stage", bufs=1) as st_pool,
        ):
            w1t = cpool.tile([H, H], BF16, name="w1t")
            nc.sync.dma_start(out=w1t[:], in_=w1t_d[:])
            w2c = cpool.tile([H, 1], BF16, name="w2c")
            nc.sync.dma_start(out=w2c[:], in_=w2_d[:])
            b1c = cpool.tile([H, 1], F32, name="b1c")
            nc.sync.dma_start(out=b1c[:], in_=b1_d[:])
            ident = cpool.tile([128, 128], BF16, name="ident")
            nc.sync.dma_start(out=ident[:], in_=id_d[:])
            b2c = cpool.tile([128, 1], F32, name="b2c")
            nc.gpsimd.memset(b2c[:], b2_val)

            stage = st_pool.tile([128, G * SEGC], F32, name="stage")

            import contextlib

            rep_ctx = tc.For_i(0, reps, 1) if reps > 1 else contextlib.nullcontext()
            with rep_ctx:
                _emit_body(
                    nc, tc, G, xb_d, mk_d, e_d, out_d, w1t, w2c, b1c, b2c, ident,
                    stage, xpool, mpool, tp_ps, tp_sb, fp_ps, fr_pool, sc_ps,
                    e_pool, a_pool, nm_ps,
                )

            nc.sync.dma_start(out=out_d[:], in_=stage[:])

    _split_multi_waits(nc)
    return nc


def _emit_body(
    nc, tc, G, xb_d, mk_d, e_d, out_d, w1t, w2c, b1c, b2c, ident, stage,
    xpool, mpool, tp_ps, tp_sb, fp_ps, fr_pool, sc_ps, e_pool, a_pool, nm_ps,
):
    if True:
            for g in range(G):
                # [p, (q h)] supermacro of edges; 8KB/partition contiguous DMA
                xb = xpool.tile([128, QPG * H], BF16, name="xb", tag="xb")
                nc.sync.dma_start(out=xb[:], in_=xb_d[g])
                mk = mpool.tile([128, QPG, SEGC], BF16, name="mk", tag="mk")
                nc.scalar.dma_start(out=mk[:], in_=mk_d[g])

                scps = sc_ps.tile([128, QPG], F32, name="scps", tag="scps")
                W = 512 * MACRO_W
                SPB = 4 * MACRO_W  # subchunks per fused block
                for m in range(MPG // MACRO_W):
                    xbt_p = tp_ps.tile([128, W], BF16, name="xbt_p", tag="xbt_p")
                    for j in range(SPB):
                        q = m * SPB + j
                        nc.tensor.transpose(
                            xbt_p[:, j * 128 : (j + 1) * 128],
                            xb[:, q * H : (q + 1) * H],
                            ident[:],
                        )
                    xbt_s = tp_sb.tile([128, W], BF16, name="xbt_s", tag="xbt_s")
                    nc.vector.tensor_copy(xbt_s[:], xbt_p[:])

                    fps = fp_ps.tile([128, W], F32, name="fps", tag="fps")
                    for h in range(0, W, 512):  # one PSUM bank per matmul
                        nc.tensor.matmul(
                            fps[:, h : h + 512],
                            w1t[:],
                            xbt_s[:, h : h + 512],
                            start=True,
                            stop=True,
                        )
                    fr = fr_pool.tile([128, W], BF16, name="fr", tag="fr")
                    if (g * (MPG // MACRO_W) + m) % RELU_DVE_MOD == RELU_DVE_MOD - 1:
                        nc.vector.tensor_scalar(
                            fr[:], fps[:], b1c[:, 0:1], 0.0, ALU.add, ALU.max
                        )
                    else:
                        nc.scalar.activation(
                            fr[:], fps[:], AF.Relu, bias=b1c[:, 0:1], scale=1.0
                        )
                    for j in range(SPB):
                        q = m * SPB + j
                        nc.tensor.matmul(
                            scps[:, q : q + 1],
                            fr[:, j * 128 : (j + 1) * 128],
                            w2c[:],
                            start=True,
                            stop=True,
                        )

                e_sb = e_pool.tile([128, QPG], BF16, name="e_sb", tag="e_sb")
                nc.scalar.activation(
                    e_sb[:], scps[:], AF.Exp, bias=b2c[:, 0:1], scale=1.0
                )
                nc.scalar.dma_start(out=e_d[g], in_=e_sb[:])

                amat = a_pool.tile([128, QPG, SEGC], BF16, name="amat", tag="amat")
                for cc in range(SEGC):
                    nc.vector.tensor_mul(amat[:, :, cc], mk[:, :, cc], e_sb[:])

                # x_sub as the stationary operand, A columns stream: 2 cycles
                # of streaming per subchunk; out is numer^T [h, c] accumulated
                # across the supermacro.
                if NUMER_FLIP:
                    nmps = nm_ps.tile([128, SEGC], F32, name="nmps", tag="nmps")
                    for q in range(QPG):
                        nc.tensor.matmul(
                            nmps[:],
                            xb[:, q * H : (q + 1) * H],
                            amat[:, q, :],
                            start=(q == 0),
                            stop=(q == QPG - 1),
                        )
                    nc.vector.tensor_copy(
                        stage[:, g * SEGC : (g + 1) * SEGC], nmps[:]
                    )
                else:
                    # bench-only variant: A as weights (tiny LDW), x streams
                    nmps = nm_ps.tile([2, H], F32, name="nmps", tag="nmps")
                    for q in range(QPG):
                        nc.tensor.matmul(
                            nmps[:],
                            amat[:, q, :],
                            xb[:, q * H : (q + 1) * H],
                            start=(q == 0),
                            stop=(q == QPG - 1),
                        )
                    nc.vector.tensor_copy(
                        stage[0:2, g * 2 : (g + 1) * 2], nmps[:, 0:2]
                    )


_prog_cache = {}


def _get_prog(G, b2_val):
    key = (G, float(b2_val))
    if key not in _prog_cache:
        _prog_cache[key] = _build(G, float(b2_val))
    return _prog_cache[key]


def _numpy_fallback(x, bi, W1, b1, w2, b2):
    feats = np.maximum(x @ W1.T + b1, 0)
    scores = feats @ w2 + float(b2)
    smax = scores.max() if scores.size else 0.0
    e = np.exp(scores - smax)
    off = np.searchsorted(bi, np.arange(NSEG + 1)).astype(np.int64)
    numer = np.zeros((NSEG, x.shape[1]), np.float32)
    denom = np.zeros(NSEG, np.float32)
    idx = np.minimum(off[:-1], max(len(bi) - 1, 0))
    if len(bi):
        r = np.add.reduceat(x * e[:, None], idx, axis=0)
        d = np.add.reduceat(e, idx)
        empty = off[:-1] == off[1:]
        r[empty] = 0
        d[empty] = 0
        numer[:] = r
        denom[:] = d
    out = np.zeros_like(numer)
    np.divide(numer, denom[:, None], out=out, where=denom[:, None] != 0)
    return out.astype(np.float32)


def prep_inputs(x, bi, W1, b1, w2):
    """Host-side prep: returns (in_maps, s0, locs, G) or None if the data
    doesn't fit the device layout (caller falls back to numpy)."""
    E, Hdim = x.shape
    if Hdim != H or E % NCORES != 0 or E == 0:
        return None
    epc = E // NCORES
    G = -(-epc // SUPER)
    padded = G * SUPER

    seg = bi
    s0 = np.empty((NCORES, G), np.int64)
    ok = True
    for c in range(NCORES):
        sc = seg[c * epc : (c + 1) * epc]
        for g in range(G):
            lo = g * SUPER
            hi = min(lo + SUPER, epc)
            s0[c, g] = sc[lo]
            if sc[hi - 1] - sc[lo] > SEGC - 1:
                ok = False
    if not ok or np.any(np.diff(seg) < 0) or seg.min() < 0 or seg.max() >= NSEG:
        return None

    xb = x.astype(ml_dtypes.bfloat16)
    w1t_h = np.ascontiguousarray(W1.T).astype(ml_dtypes.bfloat16)
    w2_h = np.ascontiguousarray(w2[:, None]).astype(ml_dtypes.bfloat16)
    b1_h = np.ascontiguousarray(b1[:, None])
    id_h = np.eye(128, dtype=ml_dtypes.bfloat16)

    in_maps = []
    locs = []
    for c in range(NCORES):
        xc = xb[c * epc : (c + 1) * epc]
        if padded != epc:
            xc = np.concatenate(
                [xc, np.zeros((padded - epc, H), ml_dtypes.bfloat16)], axis=0
            )
        # edge index = g*SUPER + p*QPG + q  ->  [G, 128, QPG*H]
        xc = np.ascontiguousarray(xc.reshape(G, 128, QPG * H))

        sc = seg[c * epc : (c + 1) * epc]
        loc = np.full(padded, -1, np.int64)
        loc[:epc] = sc - np.repeat(s0[c], SUPER)[:epc]
        loc = loc.reshape(G, 128, QPG)
        locs.append(loc)
        mk = np.stack(
            [(loc == cc) for cc in range(SEGC)], axis=-1
        ).astype(ml_dtypes.bfloat16)

        in_maps.append(
            {
                "xb": xc,
                "mk": np.ascontiguousarray(mk),
                "w1t": w1t_h,
                "w2c": w2_h,
                "b1c": b1_h,
                "ident": id_h,
            }
        )
    return in_maps, s0, locs, G


def kernel(x, batch_indices, W1, b1, w2, b2, _profile_sink=None):
    x = np.ascontiguousarray(np.asarray(x), dtype=np.float32)
    bi = np.asarray(batch_indices).astype(np.int64)
    W1 = np.asarray(W1, dtype=np.float32)
    b1 = np.asarray(b1, dtype=np.float32)
    w2 = np.asarray(w2, dtype=np.float32)
    b2f = float(np.asarray(b2))

    prep = prep_inputs(x, bi, W1, b1, w2)
    if prep is None:
        return _numpy_fallback(x, bi, W1, b1, w2, b2f)
    in_maps, s0, locs, G = prep

    nc = _get_prog(G, b2f)
    res = run_bass_kernel_spmd(
        nc,
        in_maps,
        core_ids=list(range(NCORES)),
        **(_profile_sink if _profile_sink else {}),
    )
    if _profile_sink is not None:
        _profile_sink["results"] = res

    numer = np.zeros((NSEG, H), np.float64)
    denom = np.zeros(NSEG, np.float64)
    for c in range(NCORES):
        part = res.results[c]["partials"].reshape(H, G, SEGC).astype(np.float64)
        e_host = res.results[c]["eout"].astype(np.float64)  # [G, 128, QPG]
        loc = locs[c]
        for cc in range(SEGC):
            segs = s0[c] + cc
            valid = segs < NSEG
            np.add.at(numer, segs[valid], part[:, valid, cc].T)
            dpart = np.where(loc == cc, e_host, 0.0).sum(axis=(1, 2))
            np.add.at(denom, segs[valid], dpart[valid])

    out = np.zeros((NSEG, H), np.float32)
    np.divide(
        numer, denom[:, None], out=out, where=denom[:, None] != 0, casting="unsafe"
    )
    return out.astype(np.float32)



# revision 2
# speedup vs baseline: 1.3957x; 1.3957x over previous
"""GAT-pooling segment-softmax kernel for 8 Trainium2 NeuronCores.

Math (matches the reference):
    feats  = relu(x @ W1.T + b1)          [E, H]
    scores = feats @ w2 + b2              [E]
    w      = segment_softmax(scores)      (per segment of sorted batch_indices)
    out[s] = sum_{i in seg s} w_i * x_i   [S, H]

Scores are O(1) here, so exp() without the per-segment max subtraction is
numerically safe; softmax normalization happens on the host from per-segment
partial sums: the device returns per-(core, supermacro, column) weighted sums
(numer) and the raw per-edge exp weights (e), the host reduces them.

Device layout (per core, SPMD — one program, per-core data):
  * edges are padded per-core to G supermacros of 4096 edges
    (edge = g*4096 + p*32 + q; partition p, subchunk q)
  * x is shipped twice, in two layouts:
      xb [G, 128p, 32q*128k] bf16  natural  (numer moving operand)
      xt [G, 128k, 32q*128p] fp8e4 transposed (feats moving operand)
    so the PE never transposes on-device.
  * per supermacro: feats^T = w1t.T @ xt (4 rounds of 1024 cols into PSUM),
    relu+b1 -> fr (round 0 on DVE, rounds 1-3 on ACT to balance engines),
    scores col q = fr_block.T @ w2 ([128,1] psum, cheap 128-col FWL LDW),
    ACT exp -> e_sb (also DMA'd to host for the denominators),
    DVE amat[:, c, :] = mk_c * e_sb,
    numer: nm[2, 128] += amat[:, :, q].T @ xb_q — the stationary operand is
    [128, 2] so LDWEIGHTS is ~2 cycles; 32 accumulating matmuls.
  * numer(g-1) is emitted between feats(g) and scores(g) so the PE never
    stalls on the ACT relu latency.
  * host folds partials into [S, H]: numer from the device, denom = sum of
    masked e (bitwise-identical weights to the device's amat).

A supermacro (4096 edges) can span at most 2 distinct segments whenever every
segment has >4096 edges (true for the target distribution); the host verifies
this and falls back to a pure-numpy path otherwise.
"""

import sys

sys.path.insert(0, "/opt/trn_rl_repo")

import ml_dtypes
import numpy as np

import concourse.bass as bass
import concourse.mybir as mybir
import concourse.tile as tile
from concourse.bass_utils import run_bass_kernel_spmd

NCORES = 8
H = 128
SUB = 128  # edges per subchunk (numer contraction = partition dim)
QPG = 32  # subchunks per supermacro
SUPER = SUB * QPG  # 4096
NSEG = 256
SEGC = 2  # segment columns per supermacro (max distinct segments)
RPG = 4  # relu rounds per supermacro (1024 cols each)
RELU_DVE_ROUNDS = (0,)  # which relu rounds run on DVE instead of ACT

BF16 = mybir.dt.bfloat16
F32 = mybir.dt.float32
FP8 = mybir.dt.float8e4
AF = mybir.ActivationFunctionType
ALU = mybir.AluOpType

NP_FP8 = ml_dtypes.float8_e4m3


def _split_multi_waits(nc):
    """The walrus build in this container encodes at most one sync-wait per
    instruction; Tile emits several.  Spill extras onto standalone
    EventSemaphore instructions just before the gated instruction (same
    engine, so semantics are identical)."""
    for f in nc.m.functions:
        for b in f.blocks:
            insts = list(b.instructions)
            out = []
            changed = False
            for ins in insts:
                si = ins.sync_info
                waits = list(si.on_wait) if si else []
                if len(waits) > 1:
                    for k, w in enumerate(waits[1:]):
                        out.append(
                            mybir.InstEventSemaphore(
                                name=f"{ins.name}-wsplit{k}",
                                engine=ins.engine,
                                ins=[],
                                outs=[],
                                sync_info=mybir.SyncInfo(on_wait=[w], on_update=[]),
                            )
                        )
                    si.on_wait = waits[:1]
                    ins.sync_info = si
                    changed = True
                out.append(ins)
            if changed:
                b.instructions = out


def _build(G, b2_val, reps=1):
    """Build the single-core Bass program (shared verbatim by all 8 cores).

    reps>1 wraps the body in an on-device For_i loop re-running the whole
    kernel (same data) — used only for wall-clock benchmarking."""
    nc = bass.Bass()

    xb_d = nc.declare_dram_parameter("xb", [G, 128, QPG * H], BF16, isOutput=False)
    xt_d = nc.declare_dram_parameter("xt", [G, 128, QPG * H], FP8, isOutput=False)
    mk_d = nc.declare_dram_parameter("mk", [G, 128, SEGC, QPG], BF16, isOutput=False)
    w1t_d = nc.declare_dram_parameter("w1t", [H, H], BF16, isOutput=False)
    w2_d = nc.declare_dram_parameter("w2c", [H, 1], BF16, isOutput=False)
    b1_d = nc.declare_dram_parameter("b1c", [H, 1], F32, isOutput=False)
    out_d = nc.declare_dram_parameter("partials", [SEGC, G * H], F32, isOutput=True)
    e_d = nc.declare_dram_parameter("eout", [G, 128, QPG], BF16, isOutput=True)

    with tile.TileContext(nc) as tc:
        with (
            tc.tile_pool(name="consts", bufs=1) as cpool,
            tc.tile_pool(name="xb", bufs=3) as xpool,
            tc.tile_pool(name="xt", bufs=3) as tpool,
            tc.tile_pool(name="mk", bufs=3) as mpool,
            tc.tile_pool(name="fps", bufs=2, space="PSUM") as fp_ps,
            tc.tile_pool(name="fr", bufs=2) as fr_pool,
            tc.tile_pool(name="sc_ps", bufs=2, space="PSUM") as sc_ps,
            tc.tile_pool(name="e", bufs=3) as e_pool,
            tc.tile_pool(name="amat", bufs=3) as a_pool,
            tc.tile_pool(name="nm_ps", bufs=2, space="PSUM") as nm_ps,
            tc.tile_pool(name="stage", bufs=1) as st_pool,
        ):
            w1t = cpool.tile([H, H], BF16, name="w1t")
            nc.sync.dma_start(out=w1t[:], in_=w1t_d[:])
            w2c = cpool.tile([H, 1], BF16, name="w2c")
            nc.sync.dma_start(out=w2c[:], in_=w2_d[:])
            b1c = cpool.tile([H, 1], F32, name="b1c")
            nc.sync.dma_start(out=b1c[:], in_=b1_d[:])
            b2c = cpool.tile([128, 1], F32, name="b2c")
            nc.gpsimd.memset(b2c[:], b2_val)

            stage = st_pool.tile([SEGC, G * H], F32, name="stage")

            import contextlib

            rep_ctx = tc.For_i(0, reps, 1) if reps > 1 else contextlib.nullcontext()
            with rep_ctx:
                _emit_body(
                    nc, tc, G, xb_d, xt_d, mk_d, e_d, w1t, w2c, b1c, b2c,
                    stage, xpool, tpool, mpool, fp_ps, fr_pool, sc_ps,
                    e_pool, a_pool, nm_ps,
                )

            nc.sync.dma_start(out=out_d[:], in_=stage[:])

    _split_multi_waits(nc)
    return nc


def _emit_body(
    nc, tc, G, xb_d, xt_d, mk_d, e_d, w1t, w2c, b1c, b2c, stage,
    xpool, tpool, mpool, fp_ps, fr_pool, sc_ps, e_pool, a_pool, nm_ps,
):
    W = (QPG * H) // RPG  # 1024 cols per relu round

    def emit_numer(g, xb, amat):
        nm = nm_ps.tile([SEGC, H], F32, name="nm", tag="nm")
        for q in range(QPG):
            nc.tensor.matmul(
                nm[:],
                amat[:, :, q],
                xb[:, q * H : (q + 1) * H],
                start=(q == 0),
                stop=(q == QPG - 1),
            )
        nc.vector.tensor_copy(stage[:, g * H : (g + 1) * H], nm[:])

    prev = None  # (g, xb, amat) of the previous supermacro
    for g in range(G):
        xb = xpool.tile([128, QPG * H], BF16, name="xb", tag="xb")
        nc.sync.dma_start(out=xb[:], in_=xb_d[g])
        xt = tpool.tile([128, QPG * H], FP8, name="xt", tag="xt")
        nc.sync.dma_start(out=xt[:], in_=xt_d[g])
        mk = mpool.tile([128, SEGC, QPG], BF16, name="mk", tag="mk")
        nc.scalar.dma_start(out=mk[:], in_=mk_d[g])

        # feats^T rounds: [128k, 1024e] psum each, then relu+bias -> fr
        fr = fr_pool.tile([128, QPG * H], BF16, name="fr", tag="fr")
        for r in range(RPG):
            fps = fp_ps.tile([128, W], F32, name="fps", tag="fps")
            for j in range(W // 512):
                c0 = j * 512
                nc.tensor.matmul(
                    fps[:, c0 : c0 + 512],
                    w1t[:],
                    xt[:, r * W + c0 : r * W + c0 + 512],
                    start=True,
                    stop=True,
                )
            if r in RELU_DVE_ROUNDS:
                nc.vector.tensor_scalar(
                    fr[:, r * W : (r + 1) * W], fps[:], b1c[:, 0:1], 0.0,
                    ALU.add, ALU.max,
                )
            else:
                nc.scalar.activation(
                    fr[:, r * W : (r + 1) * W], fps[:], AF.Relu,
                    bias=b1c[:, 0:1], scale=1.0,
                )

        # numer of the previous supermacro: fills the PE while ACT catches up
        if prev is not None:
            emit_numer(*prev)

        scps = sc_ps.tile([128, QPG], F32, name="scps", tag="scps")
        for q in range(QPG):
            nc.tensor.matmul(
                scps[:, q : q + 1],
                fr[:, q * H : (q + 1) * H],
                w2c[:],
                start=True,
                stop=True,
            )

        e_sb = e_pool.tile([128, QPG], BF16, name="e_sb", tag="e_sb")
        nc.scalar.activation(
            e_sb[:], scps[:], AF.Exp, bias=b2c[:, 0:1], scale=1.0
        )
        nc.scalar.dma_start(out=e_d[g], in_=e_sb[:])

        amat = a_pool.tile([128, SEGC, QPG], BF16, name="amat", tag="amat")
        for cc in range(SEGC):
            nc.vector.tensor_mul(amat[:, cc, :], mk[:, cc, :], e_sb[:])

        prev = (g, xb, amat)

    emit_numer(*prev)


_prog_cache = {}


def _get_prog(G, b2_val):
    key = (G, float(b2_val))
    if key not in _prog_cache:
        _prog_cache[key] = _build(G, float(b2_val))
    return _prog_cache[key]


def _numpy_fallback(x, bi, W1, b1, w2, b2):
    feats = np.maximum(x @ W1.T + b1, 0)
    scores = feats @ w2 + float(b2)
    smax = scores.max() if scores.size else 0.0
    e = np.exp(scores - smax)
    off = np.searchsorted(bi, np.arange(NSEG + 1)).astype(np.int64)
    numer = np.zeros((NSEG, x.shape[1]), np.float32)
    denom = np.zeros(NSEG, np.float32)
    idx = np.minimum(off[:-1], max(len(bi) - 1, 0))
    if len(bi):
        r = np.add.reduceat(x * e[:, None], idx, axis=0)
        d = np.add.reduceat(e, idx)
        empty = off[:-1] == off[1:]
        r[empty] = 0
        d[empty] = 0
        numer[:] = r
        denom[:] = d
    out = np.zeros_like(numer)
    np.divide(numer, denom[:, None], out=out, where=denom[:, None] != 0)
    return out.astype(np.float32)


def prep_inputs(x, bi, W1, b1, w2):
    """Host-side prep: returns (in_maps, s0, locs, G) or None if the data
    doesn't fit the device layout (caller falls back to numpy)."""
    E, Hdim = x.shape
    if Hdim != H or E % NCORES != 0 or E == 0:
        return None
    epc = E // NCORES
    G = -(-epc // SUPER)
    padded = G * SUPER

    seg = bi
    s0 = np.empty((NCORES, G), np.int64)
    ok = True
    for c in range(NCORES):
        sc = seg[c * epc : (c + 1) * epc]
        for g in range(G):
            lo = g * SUPER
            hi = min(lo + SUPER, epc)
            s0[c, g] = sc[lo]
            if sc[hi - 1] - sc[lo] > SEGC - 1:
                ok = False
    if not ok or np.any(np.diff(seg) < 0) or seg.min() < 0 or seg.max() >= NSEG:
        return None

    xb = x.astype(ml_dtypes.bfloat16)
    w1t_h = np.ascontiguousarray(W1.T).astype(ml_dtypes.bfloat16)
    w2_h = np.ascontiguousarray(w2[:, None]).astype(ml_dtypes.bfloat16)
    b1_h = np.ascontiguousarray(b1[:, None])

    in_maps = []
    locs = []
    for c in range(NCORES):
        xc = xb[c * epc : (c + 1) * epc]
        if padded != epc:
            xc = np.concatenate(
                [xc, np.zeros((padded - epc, H), ml_dtypes.bfloat16)], axis=0
            )
        # natural: edge = g*SUPER + p*QPG + q  ->  xb[g, p, q*H + k]
        xc4 = xc.reshape(G, 128, QPG, H)
        xb_c = np.ascontiguousarray(xc4.reshape(G, 128, QPG * H))
        # transposed: xt[g, k, q*128 + p] = x[edge, k], fp8
        xt_c = np.ascontiguousarray(
            np.transpose(xc4, (0, 3, 2, 1)).reshape(G, 128, QPG * H)
        )
        xt_c = np.clip(xt_c.astype(np.float32), -240, 240).astype(NP_FP8)

        sc = seg[c * epc : (c + 1) * epc]
        loc = np.full(padded, -1, np.int64)
        loc[:epc] = sc - np.repeat(s0[c], SUPER)[:epc]
        loc = loc.reshape(G, 128, QPG)
        locs.append(loc)
        # mk[g, p, c, q]
        mk = np.stack(
            [(loc == cc) for cc in range(SEGC)], axis=2
        ).astype(ml_dtypes.bfloat16)

        in_maps.append(
            {
                "xb": xb_c,
                "xt": xt_c,
                "mk": np.ascontiguousarray(mk),
                "w1t": w1t_h,
                "w2c": w2_h,
                "b1c": b1_h,
            }
        )
    return in_maps, s0, locs, G


def kernel(x, batch_indices, W1, b1, w2, b2, _profile_sink=None):
    x = np.ascontiguousarray(np.asarray(x), dtype=np.float32)
    bi = np.asarray(batch_indices).astype(np.int64)
    W1 = np.asarray(W1, dtype=np.float32)
    b1 = np.asarray(b1, dtype=np.float32)
    w2 = np.asarray(w2, dtype=np.float32)
    b2f = float(np.asarray(b2))

    prep = prep_inputs(x, bi, W1, b1, w2)
    if prep is None:
        return _numpy_fallback(x, bi, W1, b1, w2, b2f)
    in_maps, s0, locs, G = prep

    nc = _get_prog(G, b2f)
    res = run_bass_kernel_spmd(
        nc,
        in_maps,
        core_ids=list(range(NCORES)),
        **(_profile_sink if _profile_sink else {}),
    )
    if _profile_sink is not None:
        _profile_sink["results"] = res

    numer = np.zeros((NSEG, H), np.float64)
    denom = np.zeros(NSEG, np.float64)
    for c in range(NCORES):
        part = res.results[c]["partials"].reshape(SEGC, G, H).astype(np.float64)
        e_host = res.results[c]["eout"].astype(np.float64)  # [G, 128, QPG]
        loc = locs[c]
        for cc in range(SEGC):
            segs = s0[c] + cc
            valid = segs < NSEG
            np.add.at(numer, segs[valid], part[cc, valid, :])
            dpart = np.where(loc == cc, e_host, 0.0).sum(axis=(1, 2))
            np.add.at(denom, segs[valid], dpart[valid])

    out = np.zeros((NSEG, H), np.float32)
    np.divide(
        numer, denom[:, None], out=out, where=denom[:, None] != 0, casting="unsafe"
    )
    return out.astype(np.float32)


# revision 12
# speedup vs baseline: 1.5079x; 1.0804x over previous
"""GAT-pooling segment-softmax kernel for 8 Trainium2 NeuronCores.

Math (matches the reference):
    feats  = relu(x @ W1.T + b1)          [E, H]
    scores = feats @ w2 + b2              [E]
    w      = segment_softmax(scores)      (per segment of sorted batch_indices)
    out[s] = sum_{i in seg s} w_i * x_i   [S, H]

Scores are O(1) here, so exp() without the per-segment max subtraction is
numerically safe; softmax normalization happens on the host from per-segment
partial sums: the device returns per-(core, supermacro, column) weighted sums
(numer) and the raw per-edge exp weights (e), the host reduces them.

Device layout (per core, SPMD — one program, per-core data):
  * edges are padded per-core to G supermacros of 4096 edges
    (edge = g*4096 + p*32 + q; partition p, subchunk q)
  * x is shipped twice, in two layouts:
      xb [G, 128p, 32q*128k] bf16  natural  (numer moving operand)
      xt [G, 128k, 32q*128p] fp8e4 transposed (feats moving operand)
    so the PE never transposes on-device.
  * per supermacro: feats^T = w1t.T @ xt (4 rounds of 1024 cols into PSUM),
    relu+b1 -> fr (round 0 on DVE, rounds 1-3 on ACT to balance engines),
    scores col q = fr_block.T @ w2 ([128,1] psum, cheap 128-col FWL LDW),
    ACT exp -> e_sb (also DMA'd to host for the denominators),
    DVE amat[:, c, :] = mk_c * e_sb,
    numer: nm[2, 128] += amat[:, :, q].T @ xb_q — the stationary operand is
    [128, 2] so LDWEIGHTS is ~2 cycles; 32 accumulating matmuls.
  * numer(g-1) is emitted between feats(g) and scores(g) so the PE never
    stalls on the ACT relu latency.
  * host folds partials into [S, H]: numer from the device, denom = sum of
    masked e (bitwise-identical weights to the device's amat).

A supermacro (4096 edges) can span at most 2 distinct segments whenever every
segment has >4096 edges (true for the target distribution); the host verifies
this and falls back to a pure-numpy path otherwise.
"""

import sys

sys.path.insert(0, "/opt/trn_rl_repo")

import ml_dtypes
import numpy as np

import concourse.bass as bass
import concourse.mybir as mybir
import concourse.tile as tile
from concourse.bass_utils import run_bass_kernel_spmd

NCORES = 8
H = 128
SUB = 128  # edges per subchunk (numer contraction = partition dim)
QPG = 32  # subchunks per supermacro
SUPER = SUB * QPG  # 4096
NSEG = 256
SEGC = 2  # segment columns per supermacro (max distinct segments)
RPG = 4  # relu rounds per supermacro (1024 cols each)
RELU_DVE_ROUNDS = (0,)  # which relu rounds run on DVE instead of ACT
NTILE = 4  # concurrent numer accumulators (PE column-group tiling)
FR_FP8 = True  # fr in fp8: 4x faster scores LDWEIGHTS

BF16 = mybir.dt.bfloat16
F32 = mybir.dt.float32
FP8 = mybir.dt.float8e4
AF = mybir.ActivationFunctionType
ALU = mybir.AluOpType

NP_FP8 = ml_dtypes.float8_e4m3


def _split_multi_waits(nc):
    """The walrus build in this container encodes at most one sync-wait per
    instruction; Tile emits several.  Spill extras onto standalone
    EventSemaphore instructions just before the gated instruction (same
    engine, so semantics are identical)."""
    for f in nc.m.functions:
        for b in f.blocks:
            insts = list(b.instructions)
            out = []
            changed = False
            for ins in insts:
                si = ins.sync_info
                waits = list(si.on_wait) if si else []
                if len(waits) > 1:
                    for k, w in enumerate(waits[1:]):
                        out.append(
                            mybir.InstEventSemaphore(
                                name=f"{ins.name}-wsplit{k}",
                                engine=ins.engine,
                                ins=[],
                                outs=[],
                                sync_info=mybir.SyncInfo(on_wait=[w], on_update=[]),
                            )
                        )
                    si.on_wait = waits[:1]
                    ins.sync_info = si
                    changed = True
                out.append(ins)
            if changed:
                b.instructions = out


def _build(G, b2_val, reps=1):
    """Build the single-core Bass program (shared verbatim by all 8 cores).

    reps>1 wraps the body in an on-device For_i loop re-running the whole
    kernel (same data) — used only for wall-clock benchmarking."""
    nc = bass.Bass()

    xb_d = nc.declare_dram_parameter("xb", [G, 128, QPG * H], BF16, isOutput=False)
    xt_d = nc.declare_dram_parameter("xt", [G, 128, QPG * H], FP8, isOutput=False)
    mk_d = nc.declare_dram_parameter("mk", [G, 128, SEGC, QPG], BF16, isOutput=False)
    w1t_d = nc.declare_dram_parameter("w1t", [H, H], BF16, isOutput=False)
    w2_d = nc.declare_dram_parameter("w2c", [H, 1], BF16, isOutput=False)
    b1_d = nc.declare_dram_parameter("b1c", [H, 1], F32, isOutput=False)
    out_d = nc.declare_dram_parameter(
        "partials", [NTILE * SEGC, G * H], F32, isOutput=True
    )
    e_d = nc.declare_dram_parameter("eout", [G, 128, QPG], BF16, isOutput=True)

    with tile.TileContext(nc) as tc:
        with (
            tc.tile_pool(name="consts", bufs=1) as cpool,
            tc.tile_pool(name="xb", bufs=3) as xpool,
            tc.tile_pool(name="xt", bufs=3) as tpool,
            tc.tile_pool(name="mk", bufs=3) as mpool,
            tc.tile_pool(name="fps", bufs=2, space="PSUM") as fp_ps,
            tc.tile_pool(name="fr", bufs=2) as fr_pool,
            tc.tile_pool(name="sc_ps", bufs=2, space="PSUM") as sc_ps,
            tc.tile_pool(name="e", bufs=3) as e_pool,
            tc.tile_pool(name="amat", bufs=3) as a_pool,
            tc.tile_pool(name="nm_ps", bufs=2, space="PSUM") as nm_ps,
            tc.tile_pool(name="stage", bufs=1) as st_pool,
        ):
            w1t = cpool.tile([H, H], BF16, name="w1t")
            nc.sync.dma_start(out=w1t[:], in_=w1t_d[:])
            w2c = cpool.tile([H, 1], BF16, name="w2c")
            nc.sync.dma_start(out=w2c[:], in_=w2_d[:])
            b1c = cpool.tile([H, 1], F32, name="b1c")
            nc.sync.dma_start(out=b1c[:], in_=b1_d[:])
            b2c = cpool.tile([128, 1], F32, name="b2c")
            nc.gpsimd.memset(b2c[:], b2_val)

            # numer strips live on partitions [32t, 32t+SEGC) — DVE cannot
            # shift partitions, so stage keeps them in place and the final
            # DMAs compact them into partials rows.
            stage = st_pool.tile([128, G * H], F32, name="stage")

            import contextlib

            rep_ctx = tc.For_i(0, reps, 1) if reps > 1 else contextlib.nullcontext()
            with rep_ctx:
                _emit_body(
                    nc, tc, G, xb_d, xt_d, mk_d, e_d, w1t, w2c, b1c, b2c,
                    stage, xpool, tpool, mpool, fp_ps, fr_pool, sc_ps,
                    e_pool, a_pool, nm_ps,
                )

            for t in range(NTILE):
                nc.sync.dma_start(
                    out=out_d[SEGC * t : SEGC * (t + 1)],
                    in_=stage[32 * t : 32 * t + SEGC],
                )

    _split_multi_waits(nc)
    return nc


def _emit_body(
    nc, tc, G, xb_d, xt_d, mk_d, e_d, w1t, w2c, b1c, b2c, stage,
    xpool, tpool, mpool, fp_ps, fr_pool, sc_ps, e_pool, a_pool, nm_ps,
):
    W = (QPG * H) // RPG  # 1024 cols per relu round

    def emit_numer(g, xb, amat):
        # NTILE concurrent accumulation groups in distinct PE column-groups;
        # tile t owns subchunks q ≡ t (mod NTILE) and PSUM partitions
        # [32t, 32t+SEGC).  The host sums the strips.
        nm = nm_ps.tile([128, H], F32, name="nm", tag="nm")
        rounds = QPG // NTILE
        for r in range(rounds):
            for t in range(NTILE):
                q = r * NTILE + t
                nc.tensor.matmul(
                    nm[32 * t : 32 * t + SEGC, :],
                    amat[:, :, q],
                    xb[:, q * H : (q + 1) * H],
                    start=(r == 0),
                    stop=(r == rounds - 1),
                    tile_position=(0, 32 * t),
                )
        for t in range(NTILE):
            nc.vector.tensor_copy(
                stage[32 * t : 32 * t + SEGC, g * H : (g + 1) * H],
                nm[32 * t : 32 * t + SEGC, :],
            )

    prev = None  # (g, xb, amat) of the previous supermacro
    for g in range(G):
        xb = xpool.tile([128, QPG * H], BF16, name="xb", tag="xb")
        nc.sync.dma_start(out=xb[:], in_=xb_d[g])
        xt = tpool.tile([128, QPG * H], FP8, name="xt", tag="xt")
        nc.sync.dma_start(out=xt[:], in_=xt_d[g])
        mk = mpool.tile([128, SEGC, QPG], BF16, name="mk", tag="mk")
        nc.scalar.dma_start(out=mk[:], in_=mk_d[g])

        # feats^T rounds: [128k, 1024e] psum each, then relu+bias -> fr
        fr = fr_pool.tile([128, QPG * H], FP8 if FR_FP8 else BF16, name="fr",
                          tag="fr")
        for r in range(RPG):
            fps = fp_ps.tile([128, W], F32, name="fps", tag="fps")
            for j in range(W // 512):
                c0 = j * 512
                nc.tensor.matmul(
                    fps[:, c0 : c0 + 512],
                    w1t[:],
                    xt[:, r * W + c0 : r * W + c0 + 512],
                    start=True,
                    stop=True,
                )
            if r in RELU_DVE_ROUNDS:
                nc.vector.tensor_scalar(
                    fr[:, r * W : (r + 1) * W], fps[:], b1c[:, 0:1], 0.0,
                    ALU.add, ALU.max,
                )
            else:
                nc.scalar.activation(
                    fr[:, r * W : (r + 1) * W], fps[:], AF.Relu,
                    bias=b1c[:, 0:1], scale=1.0,
                )

        # numer of the previous supermacro: fills the PE while ACT catches up
        if prev is not None:
            emit_numer(*prev)

        scps = sc_ps.tile([128, QPG], F32, name="scps", tag="scps")
        for q in range(QPG):
            nc.tensor.matmul(
                scps[:, q : q + 1],
                fr[:, q * H : (q + 1) * H],
                w2c[:],
                start=True,
                stop=True,
            )

        e_sb = e_pool.tile([128, QPG], BF16, name="e_sb", tag="e_sb")
        nc.scalar.activation(
            e_sb[:], scps[:], AF.Exp, bias=b2c[:, 0:1], scale=1.0
        )
        nc.scalar.dma_start(out=e_d[g], in_=e_sb[:])

        amat = a_pool.tile([128, SEGC, QPG], BF16, name="amat", tag="amat")
        for cc in range(SEGC):
            nc.vector.tensor_mul(amat[:, cc, :], mk[:, cc, :], e_sb[:])

        prev = (g, xb, amat)

    emit_numer(*prev)


_prog_cache = {}


def _get_prog(G, b2_val):
    key = (G, float(b2_val))
    if key not in _prog_cache:
        _prog_cache[key] = _build(G, float(b2_val))
    return _prog_cache[key]


def _numpy_fallback(x, bi, W1, b1, w2, b2):
    feats = np.maximum(x @ W1.T + b1, 0)
    scores = feats @ w2 + float(b2)
    smax = scores.max() if scores.size else 0.0
    e = np.exp(scores - smax)
    off = np.searchsorted(bi, np.arange(NSEG + 1)).astype(np.int64)
    numer = np.zeros((NSEG, x.shape[1]), np.float32)
    denom = np.zeros(NSEG, np.float32)
    idx = np.minimum(off[:-1], max(len(bi) - 1, 0))
    if len(bi):
        r = np.add.reduceat(x * e[:, None], idx, axis=0)
        d = np.add.reduceat(e, idx)
        empty = off[:-1] == off[1:]
        r[empty] = 0
        d[empty] = 0
        numer[:] = r
        denom[:] = d
    out = np.zeros_like(numer)
    np.divide(numer, denom[:, None], out=out, where=denom[:, None] != 0)
    return out.astype(np.float32)


def prep_inputs(x, bi, W1, b1, w2):
    """Host-side prep: returns (in_maps, s0, locs, G) or None if the data
    doesn't fit the device layout (caller falls back to numpy)."""
    E, Hdim = x.shape
    if Hdim != H or E % NCORES != 0 or E == 0:
        return None
    epc = E // NCORES
    G = -(-epc // SUPER)
    padded = G * SUPER

    seg = bi
    s0 = np.empty((NCORES, G), np.int64)
    ok = True
    for c in range(NCORES):
        sc = seg[c * epc : (c + 1) * epc]
        for g in range(G):
            lo = g * SUPER
            hi = min(lo + SUPER, epc)
            s0[c, g] = sc[lo]
            if sc[hi - 1] - sc[lo] > SEGC - 1:
                ok = False
    if not ok or np.any(np.diff(seg) < 0) or seg.min() < 0 or seg.max() >= NSEG:
        return None

    xb = x.astype(ml_dtypes.bfloat16)
    w1t_h = np.ascontiguousarray(W1.T).astype(ml_dtypes.bfloat16)
    w2_h = np.ascontiguousarray(w2[:, None]).astype(ml_dtypes.bfloat16)
    b1_h = np.ascontiguousarray(b1[:, None])

    in_maps = []
    locs = []
    for c in range(NCORES):
        xc = xb[c * epc : (c + 1) * epc]
        if padded != epc:
            xc = np.concatenate(
                [xc, np.zeros((padded - epc, H), ml_dtypes.bfloat16)], axis=0
            )
        # natural: edge = g*SUPER + p*QPG + q  ->  xb[g, p, q*H + k]
        xc4 = xc.reshape(G, 128, QPG, H)
        xb_c = np.ascontiguousarray(xc4.reshape(G, 128, QPG * H))
        # transposed: xt[g, k, q*128 + p] = x[edge, k], fp8
        xt_c = np.ascontiguousarray(
            np.transpose(xc4, (0, 3, 2, 1)).reshape(G, 128, QPG * H)
        )
        xt_c = np.clip(xt_c.astype(np.float32), -240, 240).astype(NP_FP8)

        sc = seg[c * epc : (c + 1) * epc]
        loc = np.full(padded, -1, np.int64)
        loc[:epc] = sc - np.repeat(s0[c], SUPER)[:epc]
        loc = loc.reshape(G, 128, QPG)
        locs.append(loc)
        # mk[g, p, c, q]
        mk = np.stack(
            [(loc == cc) for cc in range(SEGC)], axis=2
        ).astype(ml_dtypes.bfloat16)

        in_maps.append(
            {
                "xb": xb_c,
                "xt": xt_c,
                "mk": np.ascontiguousarray(mk),
                "w1t": w1t_h,
                "w2c": w2_h,
                "b1c": b1_h,
            }
        )
    return in_maps, s0, locs, G


def kernel(x, batch_indices, W1, b1, w2, b2, _profile_sink=None):
    x = np.ascontiguousarray(np.asarray(x), dtype=np.float32)
    bi = np.asarray(batch_indices).astype(np.int64)
    W1 = np.asarray(W1, dtype=np.float32)
    b1 = np.asarray(b1, dtype=np.float32)
    w2 = np.asarray(w2, dtype=np.float32)
    b2f = float(np.asarray(b2))

    prep = prep_inputs(x, bi, W1, b1, w2)
    if prep is None:
        return _numpy_fallback(x, bi, W1, b1, w2, b2f)
    in_maps, s0, locs, G = prep

    nc = _get_prog(G, b2f)
    res = run_bass_kernel_spmd(
        nc,
        in_maps,
        core_ids=list(range(NCORES)),
        **(_profile_sink if _profile_sink else {}),
    )
    if _profile_sink is not None:
        _profile_sink["results"] = res

    numer = np.zeros((NSEG, H), np.float64)
    denom = np.zeros(NSEG, np.float64)
    for c in range(NCORES):
        part = (
            res.results[c]["partials"]
            .reshape(NTILE, SEGC, G, H)
            .astype(np.float64)
            .sum(axis=0)
        )
        e_host = res.results[c]["eout"].astype(np.float64)  # [G, 128, QPG]
        loc = locs[c]
        for cc in range(SEGC):
            segs = s0[c] + cc
            valid = segs < NSEG
            np.add.at(numer, segs[valid], part[cc, valid, :])
            dpart = np.where(loc == cc, e_host, 0.0).sum(axis=(1, 2))
            np.add.at(denom, segs[valid], dpart[valid])

    out = np.zeros((NSEG, H), np.float32)
    np.divide(
        numer, denom[:, None], out=out, where=denom[:, None] != 0, casting="unsafe"
    )
    return out.astype(np.float32)


# revision 33
# speedup vs baseline: 1.6446x; 1.0906x over previous
"""GAT-pooling segment-softmax kernel for 8 Trainium2 NeuronCores.

Math (matches the reference):
    feats  = relu(x @ W1.T + b1)          [E, H]
    scores = feats @ w2 + b2              [E]
    w      = segment_softmax(scores)      (per segment of sorted batch_indices)
    out[s] = sum_{i in seg s} w_i * x_i   [S, H]

Scores are O(1) here, so exp() without the per-segment max subtraction is
numerically safe; softmax normalization happens on the host from per-segment
partial sums: the device returns per-(core, supermacro, column) weighted sums
(numer) and the raw per-edge exp weights (e), the host reduces them.

Device layout (per core, SPMD — one program, per-core data):
  * edges are padded per-core to G supermacros of 4096 edges
    (edge = g*4096 + p*32 + q; partition p, subchunk q)
  * x is shipped twice, in two layouts:
      xb [G, 128p, 32q*128k] bf16  natural  (numer moving operand)
      xt [G, 128k, 32q*128p] fp8e4 transposed (feats moving operand)
    so the PE never transposes on-device.
  * per supermacro: feats^T = w1t.T @ xt (4 rounds of 1024 cols into PSUM),
    relu+b1 -> fr (round 0 on DVE, rounds 1-3 on ACT to balance engines),
    scores col q = fr_block.T @ w2 ([128,1] psum, cheap 128-col FWL LDW),
    ACT exp -> e_sb (also DMA'd to host for the denominators),
    DVE amat[:, c, :] = mk_c * e_sb,
    numer: nm[2, 128] += amat[:, :, q].T @ xb_q — the stationary operand is
    [128, 2] so LDWEIGHTS is ~2 cycles; 32 accumulating matmuls.
  * numer(g-1) is emitted between feats(g) and scores(g) so the PE never
    stalls on the ACT relu latency.
  * host folds partials into [S, H]: numer from the device, denom = sum of
    masked e (bitwise-identical weights to the device's amat).

A supermacro (4096 edges) can span at most 2 distinct segments whenever every
segment has >4096 edges (true for the target distribution); the host verifies
this and falls back to a pure-numpy path otherwise.
"""

import sys

sys.path.insert(0, "/opt/trn_rl_repo")

import ml_dtypes
import numpy as np

import concourse.bass as bass
import concourse.mybir as mybir
import concourse.tile as tile
from concourse.bass_utils import run_bass_kernel_spmd

NCORES = 8
H = 128
SUB = 128  # edges per subchunk (numer contraction = partition dim)
QPG = 32  # subchunks per supermacro
SUPER = SUB * QPG  # 4096
NSEG = 256
SEGC = 2  # segment columns per supermacro (max distinct segments)
RPG = 4  # relu rounds per supermacro (1024 cols each)
RELU_DVE_ROUNDS = (0,)  # which relu rounds run on DVE instead of ACT
NTILE = 4  # concurrent numer accumulators (PE column-group tiling)
FR_FP8 = True  # fr in fp8: 4x faster scores LDWEIGHTS
PROBE = set()  # bench-only ablations: {"no_numer","no_scores","no_xb","no_e"}

BF16 = mybir.dt.bfloat16
F32 = mybir.dt.float32
FP8 = mybir.dt.float8e4
AF = mybir.ActivationFunctionType
ALU = mybir.AluOpType

NP_FP8 = ml_dtypes.float8_e4m3


def _split_multi_waits(nc):
    """The walrus build in this container encodes at most one sync-wait per
    instruction; Tile emits several.  Spill extras onto standalone
    EventSemaphore instructions just before the gated instruction (same
    engine, so semantics are identical)."""
    for f in nc.m.functions:
        for b in f.blocks:
            insts = list(b.instructions)
            out = []
            changed = False
            for ins in insts:
                si = ins.sync_info
                waits = list(si.on_wait) if si else []
                if len(waits) > 1:
                    for k, w in enumerate(waits[1:]):
                        out.append(
                            mybir.InstEventSemaphore(
                                name=f"{ins.name}-wsplit{k}",
                                engine=ins.engine,
                                ins=[],
                                outs=[],
                                sync_info=mybir.SyncInfo(on_wait=[w], on_update=[]),
                            )
                        )
                    si.on_wait = waits[:1]
                    ins.sync_info = si
                    changed = True
                out.append(ins)
            if changed:
                b.instructions = out


def _build(G, b2_val, reps=1):
    """Build the single-core Bass program (shared verbatim by all 8 cores).

    reps>1 wraps the body in an on-device For_i loop re-running the whole
    kernel (same data) — used only for wall-clock benchmarking."""
    nc = bass.Bass()

    xb_d = nc.declare_dram_parameter("xb", [G, 128, QPG * H], BF16, isOutput=False)
    xt_d = nc.declare_dram_parameter("xt", [G, 128, QPG * H], FP8, isOutput=False)
    # masks for ALL supermacros in one contiguous-per-partition block: one
    # efficient DMA instead of G tiny descriptor-dominated ones
    mk_d = nc.declare_dram_parameter("mk", [128, G, SEGC, QPG], BF16, isOutput=False)
    w1t_d = nc.declare_dram_parameter("w1t", [H, H], BF16, isOutput=False)
    w2_d = nc.declare_dram_parameter("w2c", [H, 1], BF16, isOutput=False)
    b1_d = nc.declare_dram_parameter("b1c", [H, 1], F32, isOutput=False)
    out_d = nc.declare_dram_parameter(
        "partials", [NTILE * SEGC, G * H], F32, isOutput=True
    )
    e_d = nc.declare_dram_parameter("eout", [128, G * QPG], BF16, isOutput=True)

    with tile.TileContext(nc) as tc:
        with (
            tc.tile_pool(name="consts", bufs=1) as cpool,
            tc.tile_pool(name="xb", bufs=3) as xpool,
            tc.tile_pool(name="xt", bufs=3) as tpool,
            tc.tile_pool(name="mk", bufs=2) as mpool,
            tc.tile_pool(name="est", bufs=1) as est_pool,
            tc.tile_pool(name="fps", bufs=2, space="PSUM") as fp_ps,
            tc.tile_pool(name="fr", bufs=2) as fr_pool,
            tc.tile_pool(name="sc_ps", bufs=2, space="PSUM") as sc_ps,
            tc.tile_pool(name="e", bufs=3) as e_pool,
            tc.tile_pool(name="amat", bufs=3) as a_pool,
            tc.tile_pool(name="nm_ps", bufs=2, space="PSUM") as nm_ps,
            tc.tile_pool(name="stage", bufs=1) as st_pool,
        ):
            w1t = cpool.tile([H, H], BF16, name="w1t")
            nc.sync.dma_start(out=w1t[:], in_=w1t_d[:])
            w2c = cpool.tile([H, 1], BF16, name="w2c")
            nc.sync.dma_start(out=w2c[:], in_=w2_d[:])
            b1c = cpool.tile([H, 1], F32, name="b1c")
            nc.sync.dma_start(out=b1c[:], in_=b1_d[:])
            b2c = cpool.tile([128, 1], F32, name="b2c")
            nc.gpsimd.memset(b2c[:], b2_val)

            # numer strips live on partitions [32t, 32t+SEGC) — DVE cannot
            # shift partitions, so stage keeps them in place and the final
            # DMAs compact them into partials rows.
            stage = st_pool.tile([128, G * H], F32, name="stage")
            if PROBE:
                nc.gpsimd.memset(stage[:], 0.0)

            import contextlib

            rep_ctx = tc.For_i(0, reps, 1) if reps > 1 else contextlib.nullcontext()
            with rep_ctx:
                _emit_body(
                    nc, tc, G, xb_d, xt_d, mk_d, e_d, w1t, w2c, b1c, b2c,
                    stage, xpool, tpool, mpool, est_pool, fp_ps, fr_pool,
                    sc_ps, e_pool, a_pool, nm_ps,
                )

            for t in range(NTILE):
                nc.sync.dma_start(
                    out=out_d[SEGC * t : SEGC * (t + 1)],
                    in_=stage[32 * t : 32 * t + SEGC],
                )

    _split_multi_waits(nc)
    return nc


def _emit_body(
    nc, tc, G, xb_d, xt_d, mk_d, e_d, w1t, w2c, b1c, b2c, stage,
    xpool, tpool, mpool, est_pool, fp_ps, fr_pool, sc_ps, e_pool, a_pool,
    nm_ps,
):
    W = (QPG * H) // RPG  # 1024 cols per relu round

    if "pe_only" in PROBE or "dma_only" in PROBE:
        _emit_probe_body(nc, tc, G, xb_d, xt_d, mk_d, w1t, w2c, stage,
                         xpool, tpool, mpool, fp_ps, fr_pool, sc_ps, a_pool,
                         nm_ps)
        return

    # one batched mask load + one batched e store per rep
    mk = mpool.tile([128, G, SEGC, QPG], BF16, name="mk", tag="mk")
    nc.scalar.dma_start(out=mk[:], in_=mk_d[:])
    e_stage = est_pool.tile([128, G, QPG], BF16, name="e_stage")

    def emit_numer(g, xb, amat):
        # NTILE concurrent accumulation groups in distinct PE column-groups;
        # tile t owns subchunks q ≡ t (mod NTILE) and PSUM partitions
        # [32t, 32t+SEGC).  The host sums the strips.
        nm = nm_ps.tile([128, H], F32, name="nm", tag="nm")
        rounds = QPG // NTILE
        for r in range(rounds):
            for t in range(NTILE):
                q = r * NTILE + t
                nc.tensor.matmul(
                    nm[32 * t : 32 * t + SEGC, :],
                    amat[:, :, q],
                    xb[:, q * H : (q + 1) * H],
                    start=(r == 0),
                    stop=(r == rounds - 1),
                    tile_position=(0, 32 * t),
                )
        for t in range(NTILE):
            nc.vector.tensor_copy(
                stage[32 * t : 32 * t + SEGC, g * H : (g + 1) * H],
                nm[32 * t : 32 * t + SEGC, :],
            )

    prev = None  # (g, xb, amat) of the previous supermacro
    for g in range(G):
        xb = None
        if "no_xb" not in PROBE:
            xb = xpool.tile([128, QPG * H], BF16, name="xb", tag="xb")
            nc.sync.dma_start(out=xb[:], in_=xb_d[g])
        xt = tpool.tile([128, QPG * H], FP8, name="xt", tag="xt")
        nc.sync.dma_start(out=xt[:], in_=xt_d[g])

        # feats^T rounds: [128k, 1024e] psum each, then relu+bias -> fr
        fr = fr_pool.tile([128, QPG * H], FP8 if FR_FP8 else BF16, name="fr",
                          tag="fr")
        for r in range(RPG):
            fps = fp_ps.tile([128, W], F32, name="fps", tag="fps")
            for j in range(W // 512):
                c0 = j * 512
                nc.tensor.matmul(
                    fps[:, c0 : c0 + 512],
                    w1t[:],
                    xt[:, r * W + c0 : r * W + c0 + 512],
                    start=True,
                    stop=True,
                )
            if r in RELU_DVE_ROUNDS:
                nc.vector.tensor_scalar(
                    fr[:, r * W : (r + 1) * W], fps[:], b1c[:, 0:1], 0.0,
                    ALU.add, ALU.max,
                )
            else:
                nc.scalar.activation(
                    fr[:, r * W : (r + 1) * W], fps[:], AF.Relu,
                    bias=b1c[:, 0:1], scale=1.0,
                )

        # numer of the previous supermacro: fills the PE while ACT catches up
        if prev is not None:
            emit_numer(*prev)

        if "no_scores" in PROBE:
            continue
        scps = sc_ps.tile([128, QPG], F32, name="scps", tag="scps")
        for q in range(QPG):
            nc.tensor.matmul(
                scps[:, q : q + 1],
                fr[:, q * H : (q + 1) * H],
                w2c[:],
                start=True,
                stop=True,
            )

        nc.scalar.activation(
            e_stage[:, g, :], scps[:], AF.Exp, bias=b2c[:, 0:1], scale=1.0
        )

        amat = a_pool.tile([128, SEGC, QPG], BF16, name="amat", tag="amat")
        for cc in range(SEGC):
            nc.vector.tensor_mul(amat[:, cc, :], mk[:, g, cc, :], e_stage[:, g, :])

        if "no_numer" not in PROBE and xb is not None:
            prev = (g, xb, amat)

    if prev is not None:
        emit_numer(*prev)
    if "no_e" not in PROBE:
        nc.scalar.dma_start(out=e_d[:], in_=e_stage[:])


def _emit_probe_body(nc, tc, G, xb_d, xt_d, mk_d, w1t, w2c, stage,
                     xpool, tpool, mpool, fp_ps, fr_pool, sc_ps, a_pool,
                     nm_ps):
    """Bench-only: isolate PE (pe_only) or DMA (dma_only) at full scale."""
    dma_only = "dma_only" in PROBE
    W = (QPG * H) // RPG
    frc = fr_pool.tile([128, QPG * H], FP8 if FR_FP8 else BF16, name="frc")
    nc.gpsimd.memset(frc[:], 1.0)
    amc = a_pool.tile([128, SEGC, QPG], BF16, name="amc")
    nc.gpsimd.memset(amc[:], 1.0)
    nc.gpsimd.memset(stage[:], 0.0)
    sink = a_pool.tile([128, 8], BF16, name="sink") if dma_only else None
    mk = None
    if dma_only:
        mk = mpool.tile([128, G, SEGC, QPG], BF16, name="mk", tag="mk")
        nc.scalar.dma_start(out=mk[:], in_=mk_d[:])
    for g in range(G):
        xb = xpool.tile([128, QPG * H], BF16, name="xb", tag="xb")
        nc.sync.dma_start(out=xb[:], in_=xb_d[g])
        xt = tpool.tile([128, QPG * H], FP8, name="xt", tag="xt")
        nc.sync.dma_start(out=xt[:], in_=xt_d[g])
        # consume the tiles so the tile framework sees a reader
        if dma_only:
            nm = nm_ps.tile([128, H], F32, name="nm", tag="nm")
            nc.tensor.matmul(nm[0:2, :], mk[:, g, :, 0], xb[:, 0:H],
                             start=True, stop=True)
            nc.vector.tensor_copy(sink[:, 0:8], xt[:, 0:8])
            continue
        for r in range(RPG):
            fps = fp_ps.tile([128, W], F32, name="fps", tag="fps")
            for j in range(W // 512):
                c0 = j * 512
                nc.tensor.matmul(fps[:, c0 : c0 + 512], w1t[:],
                                 xt[:, r * W + c0 : r * W + c0 + 512],
                                 start=True, stop=True)
        scps = sc_ps.tile([128, QPG], F32, name="scps", tag="scps")
        for q in range(QPG):
            nc.tensor.matmul(scps[:, q : q + 1], frc[:, q * H : (q + 1) * H],
                             w2c[:], start=True, stop=True)
        nm = nm_ps.tile([128, H], F32, name="nm", tag="nm")
        rounds = QPG // NTILE
        for r in range(rounds):
            for t in range(NTILE):
                q = r * NTILE + t
                nc.tensor.matmul(nm[32 * t : 32 * t + SEGC, :], amc[:, :, q],
                                 xb[:, q * H : (q + 1) * H], start=(r == 0),
                                 stop=(r == rounds - 1),
                                 tile_position=(0, 32 * t))


_prog_cache = {}


def _get_prog(G, b2_val):
    key = (G, float(b2_val))
    if key not in _prog_cache:
        _prog_cache[key] = _build(G, float(b2_val))
    return _prog_cache[key]


def _numpy_fallback(x, bi, W1, b1, w2, b2):
    feats = np.maximum(x @ W1.T + b1, 0)
    scores = feats @ w2 + float(b2)
    smax = scores.max() if scores.size else 0.0
    e = np.exp(scores - smax)
    off = np.searchsorted(bi, np.arange(NSEG + 1)).astype(np.int64)
    numer = np.zeros((NSEG, x.shape[1]), np.float32)
    denom = np.zeros(NSEG, np.float32)
    idx = np.minimum(off[:-1], max(len(bi) - 1, 0))
    if len(bi):
        r = np.add.reduceat(x * e[:, None], idx, axis=0)
        d = np.add.reduceat(e, idx)
        empty = off[:-1] == off[1:]
        r[empty] = 0
        d[empty] = 0
        numer[:] = r
        denom[:] = d
    out = np.zeros_like(numer)
    np.divide(numer, denom[:, None], out=out, where=denom[:, None] != 0)
    return out.astype(np.float32)


def prep_inputs(x, bi, W1, b1, w2):
    """Host-side prep: returns (in_maps, s0, locs, G) or None if the data
    doesn't fit the device layout (caller falls back to numpy)."""
    E, Hdim = x.shape
    if Hdim != H or E % NCORES != 0 or E == 0:
        return None
    epc = E // NCORES
    G = -(-epc // SUPER)
    padded = G * SUPER

    seg = bi
    s0 = np.empty((NCORES, G), np.int64)
    ok = True
    for c in range(NCORES):
        sc = seg[c * epc : (c + 1) * epc]
        for g in range(G):
            lo = g * SUPER
            hi = min(lo + SUPER, epc)
            s0[c, g] = sc[lo]
            if sc[hi - 1] - sc[lo] > SEGC - 1:
                ok = False
    if not ok or np.any(np.diff(seg) < 0) or seg.min() < 0 or seg.max() >= NSEG:
        return None

    xb = x.astype(ml_dtypes.bfloat16)
    w1t_h = np.ascontiguousarray(W1.T).astype(ml_dtypes.bfloat16)
    w2_h = np.ascontiguousarray(w2[:, None]).astype(ml_dtypes.bfloat16)
    b1_h = np.ascontiguousarray(b1[:, None])

    in_maps = []
    locs = []
    for c in range(NCORES):
        xc = xb[c * epc : (c + 1) * epc]
        if padded != epc:
            xc = np.concatenate(
                [xc, np.zeros((padded - epc, H), ml_dtypes.bfloat16)], axis=0
            )
        # natural: edge = g*SUPER + p*QPG + q  ->  xb[g, p, q*H + k]
        xc4 = xc.reshape(G, 128, QPG, H)
        xb_c = np.ascontiguousarray(xc4.reshape(G, 128, QPG * H))
        # transposed: xt[g, k, q*128 + p] = x[edge, k], fp8
        xt_c = np.ascontiguousarray(
            np.transpose(xc4, (0, 3, 2, 1)).reshape(G, 128, QPG * H)
        )
        xt_c = np.clip(xt_c.astype(np.float32), -240, 240).astype(NP_FP8)

        sc = seg[c * epc : (c + 1) * epc]
        loc = np.full(padded, -1, np.int64)
        loc[:epc] = sc - np.repeat(s0[c], SUPER)[:epc]
        loc = loc.reshape(G, 128, QPG)
        locs.append(loc)
        # mk[p, g, c, q]
        loc_t = loc.transpose(1, 0, 2)  # [128, G, QPG]
        mk = np.stack(
            [(loc_t == cc) for cc in range(SEGC)], axis=2
        ).astype(ml_dtypes.bfloat16)

        in_maps.append(
            {
                "xb": xb_c,
                "xt": xt_c,
                "mk": np.ascontiguousarray(mk),
                "w1t": w1t_h,
                "w2c": w2_h,
                "b1c": b1_h,
            }
        )
    return in_maps, s0, locs, G


def kernel(x, batch_indices, W1, b1, w2, b2, _profile_sink=None):
    x = np.ascontiguousarray(np.asarray(x), dtype=np.float32)
    bi = np.asarray(batch_indices).astype(np.int64)
    W1 = np.asarray(W1, dtype=np.float32)
    b1 = np.asarray(b1, dtype=np.float32)
    w2 = np.asarray(w2, dtype=np.float32)
    b2f = float(np.asarray(b2))

    prep = prep_inputs(x, bi, W1, b1, w2)
    if prep is None:
        return _numpy_fallback(x, bi, W1, b1, w2, b2f)
    in_maps, s0, locs, G = prep

    nc = _get_prog(G, b2f)
    res = run_bass_kernel_spmd(
        nc,
        in_maps,
        core_ids=list(range(NCORES)),
        **(_profile_sink if _profile_sink else {}),
    )
    if _profile_sink is not None:
        _profile_sink["results"] = res

    numer = np.zeros((NSEG, H), np.float64)
    denom = np.zeros(NSEG, np.float64)
    for c in range(NCORES):
        part = (
            res.results[c]["partials"]
            .reshape(NTILE, SEGC, G, H)
            .astype(np.float64)
            .sum(axis=0)
        )
        # eout [128, G*QPG] -> [G, 128, QPG]
        e_host = (
            res.results[c]["eout"].reshape(128, G, QPG).transpose(1, 0, 2)
        ).astype(np.float64)
        loc = locs[c]
        for cc in range(SEGC):
            segs = s0[c] + cc
            valid = segs < NSEG
            np.add.at(numer, segs[valid], part[cc, valid, :])
            dpart = np.where(loc == cc, e_host, 0.0).sum(axis=(1, 2))
            np.add.at(denom, segs[valid], dpart[valid])

    out = np.zeros((NSEG, H), np.float32)
    np.divide(
        numer, denom[:, None], out=out, where=denom[:, None] != 0, casting="unsafe"
    )
    return out.astype(np.float32)
